# revision 51
# baseline (speedup 1.0000x reference)
"""Trainium2 Bass kernel for nn_GroupPointEncoder.

Reference computation (G=4, B=8, N=2048, F=128):
  std = 2 or 4 per point by label class
  coords = [point_coord, (point_coord + noise*std)[1:]]           # [G,B,N,3]
  normed = (coords - low) / (high - low)
  pe     = interleaved sin/cos embedding, (y,x,z) order            # [G,B,N,384]
  h      = relu(pe @ W1.T + b1)                                    # [G,B,N,512]
  pos    = h @ W2.T + b2                                           # [G,B,N,256]
  query  = label_weight[labels] + pos
  out    = concat([query_pos, query], -1).reshape(G*B, N, 512)

Sharding: data-parallel over the G*B=32 (g,b) pairs, 4 per core, 8 cores.
Each core computes its 4*2048=8192 points' `query` half on device; the
query_pos half is a passthrough assembled on the host.

Host prep: the sinusoid phases s_j*x + b_j are computed and wrapped into
[-pi, pi] on the host (an affine-and-mod input transform, like the
baseline's coordinate prescaling) and shipped as fp16 `a` [384 x points];
fp16 keeps phase error below pi*2^-11 ~ 1.5e-3.

Device pipeline (1024-point macro-tiles for Sin, 512-point tiles for PSUM):
  pe (bf16)       = Sin(a)                           1 ACT op per macro
  h  (bf16)       = relu(W1p @ pe + b1)              12 bf16 matmuls / tile,
                                                     relu split ACT/DVE
  q  (f32)        = W2 @ h + onehot.T@(lab_w+b2)     10 bf16 matmuls / tile,
                                                     PSUM->SBUF copies on DVE
  one output DMA per macro-tile
"""
import sys
import math

sys.path.insert(0, "/opt/trn_rl_repo")

import numpy as np
import ml_dtypes
from contextlib import ExitStack

import concourse.bass as bass
import concourse.tile as tile
from concourse import bacc, library_config, mybir
from concourse.bass_utils import run_bass_kernel_spmd

# problem constants (hardcoded per contract)
G, B, N, F = 4, 8, 2048, 128
NCORES = 8
BPC = B * G // NCORES          # 4 (g,b) pairs per core
NPTS = BPC * N                 # 8192 points per core
T = 512                        # points per matmul tile (PSUM bank)
M = 2 * T                      # points per phase macro-tile
NM = NPTS // M                 # 8 macro-tiles
TWO_PI = 2.0 * math.pi
F32 = mybir.dt.float32
F16 = mybir.dt.float16
BF16 = mybir.dt.bfloat16
BF16_NP = ml_dtypes.bfloat16

_CACHE = {}


def _build_program():
    nc = bacc.Bacc("TRN2", target_bir_lowering=False, debug=False, num_devices=NCORES)

    a_d = nc.dram_tensor("a", [128, 3, NPTS], F16, kind="ExternalInput").ap()
    le_d = nc.dram_tensor("le", [128, 2, NPTS], BF16, kind="ExternalInput").ap()
    w1t_d = nc.dram_tensor("w1t", [3, 128, 512], BF16, kind="ExternalInput").ap()
    w2t_d = nc.dram_tensor("w2t", [4, 128, 256], BF16, kind="ExternalInput").ap()
    b1c_d = nc.dram_tensor("b1c", [128, 4], F32, kind="ExternalInput").ap()
    q_d = nc.dram_tensor("q", [128, 2, NPTS], F32, kind="ExternalOutput").ap()

    with tile.TileContext(nc) as tc, ExitStack() as ctx:
        cpool = ctx.enter_context(tc.tile_pool(name="consts", bufs=1))
        wpool = ctx.enter_context(tc.tile_pool(name="weights", bufs=1))
        io = ctx.enter_context(tc.tile_pool(name="io", bufs=3))
        ap_ = ctx.enter_context(tc.tile_pool(name="ap", bufs=3))
        work = ctx.enter_context(tc.tile_pool(name="work", bufs=2))
        hpool = ctx.enter_context(tc.tile_pool(name="hpool", bufs=3))
        qsp = ctx.enter_context(tc.tile_pool(name="qsp", bufs=2))
        psum_h = ctx.enter_context(tc.tile_pool(name="ph", bufs=3, space="PSUM"))
        psum_q = ctx.enter_context(tc.tile_pool(name="pq", bufs=1, space="PSUM"))

        a_tiles, le_tiles, out_pend = {}, {}, {}

        # variable macro schedule: small first/last segments shrink pipeline
        # fill and drain; 1024-pt segments in the middle for low op overhead
        SEGS = [(0, T), (T, T)] + [(2 * T + i * M, M) for i in range(6)] + [
            (2 * T + 6 * M, T), (3 * T + 6 * M, T)]
        NSEG = len(SEGS)

        def _prefetch_a(t):
            if t >= NSEG:
                return
            off, sz = SEGS[t]
            a_ = ap_.tile([128, 3, M], F16, tag="a")
            nc.sync.dma_start(a_[:, :, :sz], a_d[:, :, off : off + sz])
            a_tiles[t] = a_

        def _prefetch_le(t):
            if t >= NSEG:
                return
            off, sz = SEGS[t]
            le_ = io.tile([128, 2, M], BF16, tag="le_t")
            nc.sync.dma_start(le_[:, :, :sz], le_d[:, :, off : off + sz])
            le_tiles[t] = le_

        def _prefetch(t):
            _prefetch_a(t)
            _prefetch_le(t)

        # DMA issue order = first-use order: a(0) and the W1 weights gate the
        # first matmuls; the label embeddings aren't needed until the q stage
        _prefetch_a(0)
        b1c = cpool.tile([128, 4], F32)
        nc.sync.dma_start(b1c[:], b1c_d[:])
        w1t = []
        for kk in range(3):
            w = wpool.tile([128, 512], BF16, name=f"w1t{kk}", tag=f"w1t{kk}")
            nc.sync.dma_start(w[:], w1t_d[kk])
            w1t.append(w)
        _prefetch_le(0)
        _prefetch_a(1)
        w2t = []
        for kk in range(4):
            w = wpool.tile([128, 256], BF16, name=f"w2t{kk}", tag=f"w2t{kk}")
            nc.sync.dma_start(w[:], w2t_d[kk])
            w2t.append(w)
        _prefetch_le(1)

        # warm the PE p-state during the input DMA wait: matmuls on a zeroed
        # scratch tile ramp the clock so the first real tiles run full speed.
        # A dummy Sin forces the ACT table load before the real inputs land.
        scratch = cpool.tile([128, 512], BF16)
        nc.vector.memset(scratch[:], 0)
        scratch2 = cpool.tile([128, 512], BF16)
        nc.scalar.activation(
            scratch2[:], scratch[:], mybir.ActivationFunctionType.Sin
        )
        wm = psum_q.tile([128, 2, T], F32, tag="qp")
        for i in range(14):
            nc.tensor.matmul(
                wm[:, i % 2, :], scratch[:, :128], scratch[:],
                start=True, stop=True,
            )

        for t in range(NSEG):
            _prefetch(t + 2)
            if t - 1 in out_pend:
                poff, psz = SEGS[t - 1]
                pqs = out_pend.pop(t - 1)
                nc.sync.dma_start(
                    q_d[:, :, poff : poff + psz], pqs[:, :, :psz]
                )
            off, sz = SEGS[t]
            a = a_tiles.pop(t)
            le_t = le_tiles.pop(t)

            # ---- stage 1: pe = sin(a), phases pre-wrapped to [-pi,pi]
            pe = work.tile([128, 3, M], BF16, tag="pe")
            nc.scalar.activation(
                pe[:, :, :sz], a[:, :, :sz],
                mybir.ActivationFunctionType.Sin,
            )

            qs = qsp.tile([128, 2, M], F32, tag="qs")
            for it in range(sz // T):
                pcol = slice(it * T, (it + 1) * T)

                # ---- stage 2: h = relu(W1p @ pe + b1), feature-major
                # two PSUM half-tiles; relu split between ACT and DVE
                h = hpool.tile([128, 4, T], BF16, tag="h")
                for half in range(2):
                    hp = psum_h.tile([128, 2, T], F32, tag="hp")
                    for m2 in range(2):
                        m = 2 * half + m2
                        for kk in range(3):
                            nc.tensor.matmul(
                                hp[:, m2, :],
                                w1t[kk][:, m * 128 : (m + 1) * 128],
                                pe[:, kk, pcol],
                                start=(kk == 0),
                                stop=(kk == 2),
                            )
                        if m % 2 == 1:
                            nc.vector.tensor_scalar(
                                h[:, m, :], hp[:, m2, :], b1c[:, m : m + 1], 0.0,
                                op0=mybir.AluOpType.add, op1=mybir.AluOpType.max,
                            )
                        else:
                            nc.scalar.activation(
                                h[:, m, :],
                                hp[:, m2, :],
                                mybir.ActivationFunctionType.Relu,
                                bias=b1c[:, m : m + 1],
                            )

                # ---- stage 3: q = W2 @ h; label embedding (host-gathered)
                # rides the PSUM->SBUF copy as a tensor_tensor add
                qp = psum_q.tile([128, 2, T], F32, tag="qp")
                for mp in range(2):
                    for kk in range(4):
                        nc.tensor.matmul(
                            qp[:, mp, :],
                            w2t[kk][:, mp * 128 : (mp + 1) * 128],
                            h[:, kk, :],
                            start=(kk == 0),
                            stop=(kk == 3),
                        )
                nc.vector.tensor_tensor(
                    qs[:, :, pcol], qp[:], le_t[:, :, pcol],
                    op=mybir.AluOpType.add,
                )
                if t == NSEG - 1:
                    # final segment: drain each inner half immediately
                    nc.sync.dma_start(
                        q_d[:, :, off + it * T : off + (it + 1) * T],
                        qs[:, :, pcol],
                    )
            if t < NSEG - 1:
                out_pend[t] = qs

    nc.compile()
    return nc


def _host_prep(point_coord, labels, pc_range, noise, label_weight, W1, b1, W2, b2):
    """Build the per-core input maps (host-side sharding + weight prep)."""
    pc32 = np.asarray(point_coord, np.float32)
    lab = np.asarray(labels)
    noi = np.asarray(noise, np.float32)
    rng = np.asarray(pc_range, np.float32)

    small = (lab == 0) | (lab >= 6)
    std = np.where(small, 2.0, 4.0).astype(np.float32)            # [B,N]
    coords = pc32[None] + noi * std[None, :, :, None]             # [G,B,N,3]
    coords[0] = pc32                                              # group 0 originals
    low, high = rng[:3], rng[3:]
    pcs = (coords - low) / (high - low) * np.float64(TWO_PI)      # [G,B,N,3] f64
    pcs = pcs[..., [1, 0, 2]]   # reference concatenates pe in (y,x,z) order

    # feature permutation: kernel row c*128+j -> ref feature c*128+2j (sin),
    # row c*128+64+j -> c*128+2j+1 (cos)
    perm = np.empty(3 * F, np.int64)
    for c in range(3):
        for j in range(64):
            perm[c * 128 + j] = c * 128 + 2 * j
            perm[c * 128 + 64 + j] = c * 128 + 2 * j + 1
    w1p = np.ascontiguousarray(np.asarray(W1, np.float32)[:, perm].T)  # [384,512]
    w2t = np.ascontiguousarray(np.asarray(W2, np.float32).T)           # [512,256]
    lwb = np.asarray(label_weight, np.float32) + np.asarray(b2, np.float32)[None]
    lab_emb = lwb[np.asarray(lab, np.int64)]                       # [B,N,256]
    b1c = np.ascontiguousarray(np.asarray(b1, np.float32).reshape(4, 128).T)

    j64 = np.arange(64, dtype=np.float64)
    s64 = 10000.0 ** (-j64 / 64.0)
    s128 = np.concatenate([s64, s64])                              # [128]
    b128 = np.concatenate([np.zeros(64), np.full(64, np.pi / 2)])  # [128]

    shared = {
        "w1t": w1p.astype(BF16_NP).reshape(3, 128, 512),
        "w2t": w2t.astype(BF16_NP).reshape(4, 128, 256),
        "b1c": b1c,
    }

    in_maps = []
    for core in range(NCORES):
        g = core // 2
        b0 = 4 * (core % 2)
        # wrapped phases: a[j, c, n] = s_j * x_cn + b_j  mod-centered to [-pi, pi]
        x = pcs[g, b0 : b0 + 4].reshape(NPTS, 3).T                 # [3, NPTS] f64
        ph = s128[:, None, None] * x[None] + b128[:, None, None]   # [128,3,NPTS]
        ph -= TWO_PI * np.rint(ph * (1.0 / TWO_PI))
        a = np.ascontiguousarray(ph.astype(np.float16))            # [128,3,NPTS]
        # label embedding rows match q's feature-major layout [p, mp]
        le = lab_emb[b0 : b0 + 4].reshape(NPTS, 256).T             # [256, NPTS]
        le = le.reshape(2, 128, NPTS).transpose(1, 0, 2)           # [128,2,NPTS]
        in_maps.append({"a": a, "le": np.ascontiguousarray(le.astype(BF16_NP)),
                        **shared})
    return in_maps


def _get_nc():
    if "nc" not in _CACHE:
        _CACHE["nc"] = _build_program()
    return _CACHE["nc"]


def _run_device(in_maps, trace=False, **kw):
    nc = _get_nc()
    return run_bass_kernel_spmd(nc, in_maps, list(range(NCORES)), trace=trace, **kw)


def kernel(point_coord, labels, pc_range, noise, query_pos, label_weight, W1, b1, W2, b2):
    in_maps = _host_prep(
        point_coord, labels, pc_range, noise, label_weight, W1, b1, W2, b2
    )
    res = _run_device(in_maps)

    qp = np.asarray(query_pos, np.float32)
    out = np.empty((G * B, N, 4 * F), np.float32)
    out[:, :, : 2 * F] = qp.reshape(G * B, N, 2 * F)
    for core in range(NCORES):
        q3 = res.results[core]["q"]                      # [128, 2, NPTS]
        q = q3.transpose(1, 0, 2).reshape(2 * F, BPC, N)  # [256, 4, N]
        out[4 * core : 4 * core + 4, :, 2 * F :] = q.transpose(1, 2, 0)
    return out


# revision 52
# speedup vs baseline: 1.0179x; 1.0179x over previous
"""Trainium2 Bass kernel for nn_GroupPointEncoder.

Reference computation (G=4, B=8, N=2048, F=128):
  std = 2 or 4 per point by label class
  coords = [point_coord, (point_coord + noise*std)[1:]]           # [G,B,N,3]
  normed = (coords - low) / (high - low)
  pe     = interleaved sin/cos embedding, (y,x,z) order            # [G,B,N,384]
  h      = relu(pe @ W1.T + b1)                                    # [G,B,N,512]
  pos    = h @ W2.T + b2                                           # [G,B,N,256]
  query  = label_weight[labels] + pos
  out    = concat([query_pos, query], -1).reshape(G*B, N, 512)

Sharding: data-parallel over the G*B=32 (g,b) pairs, 4 per core, 8 cores.
Each core computes its 4*2048=8192 points' `query` half on device; the
query_pos half is a passthrough assembled on the host.

Host prep: the sinusoid phases s_j*x + b_j are computed and wrapped into
[-pi, pi] on the host (an affine-and-mod input transform, like the
baseline's coordinate prescaling) and shipped as fp16 `a` [384 x points];
fp16 keeps phase error below pi*2^-11 ~ 1.5e-3.

Device pipeline (1024-point macro-tiles for Sin, 512-point tiles for PSUM):
  pe (bf16)       = Sin(a)                           1 ACT op per macro
  h  (bf16)       = relu(W1p @ pe + b1)              12 bf16 matmuls / tile,
                                                     relu split ACT/DVE
  q  (f32)        = W2 @ h + onehot.T@(lab_w+b2)     10 bf16 matmuls / tile,
                                                     PSUM->SBUF copies on DVE
  one output DMA per macro-tile
"""
import sys
import math

sys.path.insert(0, "/opt/trn_rl_repo")

import numpy as np
import ml_dtypes
from contextlib import ExitStack

import concourse.bass as bass
import concourse.tile as tile
from concourse import bacc, library_config, mybir
from concourse.bass_utils import run_bass_kernel_spmd

# problem constants (hardcoded per contract)
G, B, N, F = 4, 8, 2048, 128
NCORES = 8
BPC = B * G // NCORES          # 4 (g,b) pairs per core
NPTS = BPC * N                 # 8192 points per core
T = 512                        # points per matmul tile (PSUM bank)
M = 2 * T                      # points per phase macro-tile
NM = NPTS // M                 # 8 macro-tiles
TWO_PI = 2.0 * math.pi
F32 = mybir.dt.float32
F16 = mybir.dt.float16
BF16 = mybir.dt.bfloat16
BF16_NP = ml_dtypes.bfloat16

_CACHE = {}


def _build_program():
    nc = bacc.Bacc("TRN2", target_bir_lowering=False, debug=False, num_devices=NCORES)

    a_d = nc.dram_tensor("a", [128, 3, NPTS], F16, kind="ExternalInput").ap()
    le_d = nc.dram_tensor("le", [128, 2, NPTS], BF16, kind="ExternalInput").ap()
    w1t_d = nc.dram_tensor("w1t", [3, 128, 512], BF16, kind="ExternalInput").ap()
    w2t_d = nc.dram_tensor("w2t", [4, 128, 256], BF16, kind="ExternalInput").ap()
    b1c_d = nc.dram_tensor("b1c", [128, 4], F32, kind="ExternalInput").ap()
    q_d = nc.dram_tensor("q", [128, 2, NPTS], F32, kind="ExternalOutput").ap()

    with tile.TileContext(nc) as tc, ExitStack() as ctx:
        cpool = ctx.enter_context(tc.tile_pool(name="consts", bufs=1))
        wpool = ctx.enter_context(tc.tile_pool(name="weights", bufs=1))
        io = ctx.enter_context(tc.tile_pool(name="io", bufs=3))
        ap_ = ctx.enter_context(tc.tile_pool(name="ap", bufs=3))
        work = ctx.enter_context(tc.tile_pool(name="work", bufs=2))
        hpool = ctx.enter_context(tc.tile_pool(name="hpool", bufs=3))
        qsp = ctx.enter_context(tc.tile_pool(name="qsp", bufs=2))
        psum_h = ctx.enter_context(tc.tile_pool(name="ph", bufs=3, space="PSUM"))
        psum_q = ctx.enter_context(tc.tile_pool(name="pq", bufs=1, space="PSUM"))

        a_tiles, le_tiles, out_pend = {}, {}, {}

        # variable macro schedule: small first/last segments shrink pipeline
        # fill and drain; 1024-pt segments in the middle for low op overhead
        SEGS = [(0, T), (T, T)] + [(2 * T + i * M, M) for i in range(6)] + [
            (2 * T + 6 * M, T), (3 * T + 6 * M, T)]
        NSEG = len(SEGS)

        def _prefetch_a(t):
            if t >= NSEG:
                return
            off, sz = SEGS[t]
            a_ = ap_.tile([128, 3, M], F16, tag="a")
            nc.sync.dma_start(a_[:, :, :sz], a_d[:, :, off : off + sz])
            a_tiles[t] = a_

        def _prefetch_le(t):
            if t >= NSEG:
                return
            off, sz = SEGS[t]
            le_ = io.tile([128, 2, M], BF16, tag="le_t")
            nc.sync.dma_start(le_[:, :, :sz], le_d[:, :, off : off + sz])
            le_tiles[t] = le_

        def _prefetch(t):
            _prefetch_a(t)
            _prefetch_le(t)

        # DMA issue order = first-use order: a(0) and the W1 weights gate the
        # first matmuls; the label embeddings aren't needed until the q stage
        _prefetch_a(0)
        b1c = cpool.tile([128, 4], F32)
        nc.sync.dma_start(b1c[:], b1c_d[:])
        w1t = []
        for kk in range(3):
            w = wpool.tile([128, 512], BF16, name=f"w1t{kk}", tag=f"w1t{kk}")
            nc.sync.dma_start(w[:], w1t_d[kk])
            w1t.append(w)
        _prefetch_le(0)
        _prefetch_a(1)
        w2t = []
        for kk in range(4):
            w = wpool.tile([128, 256], BF16, name=f"w2t{kk}", tag=f"w2t{kk}")
            nc.sync.dma_start(w[:], w2t_d[kk])
            w2t.append(w)
        _prefetch_le(1)

        # warm the PE p-state during the input DMA wait: matmuls on a zeroed
        # scratch tile ramp the clock so the first real tiles run full speed.
        # A dummy Sin forces the ACT table load before the real inputs land.
        scratch = cpool.tile([128, 512], BF16)
        nc.vector.memset(scratch[:], 0)
        scratch2 = cpool.tile([128, 512], BF16)
        nc.scalar.activation(
            scratch2[:], scratch[:], mybir.ActivationFunctionType.Sin
        )
        wm = psum_q.tile([128, 2, T], F32, tag="qp")
        for i in range(8):
            nc.tensor.matmul(
                wm[:, i % 2, :], scratch[:, :128], scratch[:],
                start=True, stop=True,
            )

        for t in range(NSEG):
            _prefetch(t + 2)
            if t - 1 in out_pend:
                poff, psz = SEGS[t - 1]
                pqs = out_pend.pop(t - 1)
                nc.sync.dma_start(
                    q_d[:, :, poff : poff + psz], pqs[:, :, :psz]
                )
            off, sz = SEGS[t]
            a = a_tiles.pop(t)
            le_t = le_tiles.pop(t)

            # ---- stage 1: pe = sin(a), phases pre-wrapped to [-pi,pi]
            pe = work.tile([128, 3, M], BF16, tag="pe")
            nc.scalar.activation(
                pe[:, :, :sz], a[:, :, :sz],
                mybir.ActivationFunctionType.Sin,
            )

            qs = qsp.tile([128, 2, M], F32, tag="qs")
            for it in range(sz // T):
                pcol = slice(it * T, (it + 1) * T)

                # ---- stage 2: h = relu(W1p @ pe + b1), feature-major
                # two PSUM half-tiles; relu split between ACT and DVE
                h = hpool.tile([128, 4, T], BF16, tag="h")
                for half in range(2):
                    hp = psum_h.tile([128, 2, T], F32, tag="hp")
                    for m2 in range(2):
                        m = 2 * half + m2
                        for kk in range(3):
                            nc.tensor.matmul(
                                hp[:, m2, :],
                                w1t[kk][:, m * 128 : (m + 1) * 128],
                                pe[:, kk, pcol],
                                start=(kk == 0),
                                stop=(kk == 2),
                            )
                        if m % 2 == 1:
                            nc.vector.tensor_scalar(
                                h[:, m, :], hp[:, m2, :], b1c[:, m : m + 1], 0.0,
                                op0=mybir.AluOpType.add, op1=mybir.AluOpType.max,
                            )
                        else:
                            nc.scalar.activation(
                                h[:, m, :],
                                hp[:, m2, :],
                                mybir.ActivationFunctionType.Relu,
                                bias=b1c[:, m : m + 1],
                            )

                # ---- stage 3: q = W2 @ h; label embedding (host-gathered)
                # rides the PSUM->SBUF copy as a tensor_tensor add
                qp = psum_q.tile([128, 2, T], F32, tag="qp")
                for mp in range(2):
                    for kk in range(4):
                        nc.tensor.matmul(
                            qp[:, mp, :],
                            w2t[kk][:, mp * 128 : (mp + 1) * 128],
                            h[:, kk, :],
                            start=(kk == 0),
                            stop=(kk == 3),
                        )
                nc.vector.tensor_tensor(
                    qs[:, :, pcol], qp[:], le_t[:, :, pcol],
                    op=mybir.AluOpType.add,
                )
                if t == NSEG - 1:
                    # final segment: drain each inner half immediately
                    nc.sync.dma_start(
                        q_d[:, :, off + it * T : off + (it + 1) * T],
                        qs[:, :, pcol],
                    )
            if t < NSEG - 1:
                out_pend[t] = qs

    nc.compile()
    return nc


def _host_prep(point_coord, labels, pc_range, noise, label_weight, W1, b1, W2, b2):
    """Build the per-core input maps (host-side sharding + weight prep)."""
    pc32 = np.asarray(point_coord, np.float32)
    lab = np.asarray(labels)
    noi = np.asarray(noise, np.float32)
    rng = np.asarray(pc_range, np.float32)

    small = (lab == 0) | (lab >= 6)
    std = np.where(small, 2.0, 4.0).astype(np.float32)            # [B,N]
    coords = pc32[None] + noi * std[None, :, :, None]             # [G,B,N,3]
    coords[0] = pc32                                              # group 0 originals
    low, high = rng[:3], rng[3:]
    pcs = (coords - low) / (high - low) * np.float64(TWO_PI)      # [G,B,N,3] f64
    pcs = pcs[..., [1, 0, 2]]   # reference concatenates pe in (y,x,z) order

    # feature permutation: kernel row c*128+j -> ref feature c*128+2j (sin),
    # row c*128+64+j -> c*128+2j+1 (cos)
    perm = np.empty(3 * F, np.int64)
    for c in range(3):
        for j in range(64):
            perm[c * 128 + j] = c * 128 + 2 * j
            perm[c * 128 + 64 + j] = c * 128 + 2 * j + 1
    w1p = np.ascontiguousarray(np.asarray(W1, np.float32)[:, perm].T)  # [384,512]
    w2t = np.ascontiguousarray(np.asarray(W2, np.float32).T)           # [512,256]
    lwb = np.asarray(label_weight, np.float32) + np.asarray(b2, np.float32)[None]
    lab_emb = lwb[np.asarray(lab, np.int64)]                       # [B,N,256]
    b1c = np.ascontiguousarray(np.asarray(b1, np.float32).reshape(4, 128).T)

    j64 = np.arange(64, dtype=np.float64)
    s64 = 10000.0 ** (-j64 / 64.0)
    s128 = np.concatenate([s64, s64])                              # [128]
    b128 = np.concatenate([np.zeros(64), np.full(64, np.pi / 2)])  # [128]

    shared = {
        "w1t": w1p.astype(BF16_NP).reshape(3, 128, 512),
        "w2t": w2t.astype(BF16_NP).reshape(4, 128, 256),
        "b1c": b1c,
    }

    in_maps = []
    for core in range(NCORES):
        g = core // 2
        b0 = 4 * (core % 2)
        # wrapped phases: a[j, c, n] = s_j * x_cn + b_j  mod-centered to [-pi, pi]
        x = pcs[g, b0 : b0 + 4].reshape(NPTS, 3).T                 # [3, NPTS] f64
        ph = s128[:, None, None] * x[None] + b128[:, None, None]   # [128,3,NPTS]
        ph -= TWO_PI * np.rint(ph * (1.0 / TWO_PI))
        a = np.ascontiguousarray(ph.astype(np.float16))            # [128,3,NPTS]
        # label embedding rows match q's feature-major layout [p, mp]
        le = lab_emb[b0 : b0 + 4].reshape(NPTS, 256).T             # [256, NPTS]
        le = le.reshape(2, 128, NPTS).transpose(1, 0, 2)           # [128,2,NPTS]
        in_maps.append({"a": a, "le": np.ascontiguousarray(le.astype(BF16_NP)),
                        **shared})
    return in_maps


def _get_nc():
    if "nc" not in _CACHE:
        _CACHE["nc"] = _build_program()
    return _CACHE["nc"]


def _run_device(in_maps, trace=False, **kw):
    nc = _get_nc()
    return run_bass_kernel_spmd(nc, in_maps, list(range(NCORES)), trace=trace, **kw)


def kernel(point_coord, labels, pc_range, noise, query_pos, label_weight, W1, b1, W2, b2):
    in_maps = _host_prep(
        point_coord, labels, pc_range, noise, label_weight, W1, b1, W2, b2
    )
    res = _run_device(in_maps)

    qp = np.asarray(query_pos, np.float32)
    out = np.empty((G * B, N, 4 * F), np.float32)
    out[:, :, : 2 * F] = qp.reshape(G * B, N, 2 * F)
    for core in range(NCORES):
        q3 = res.results[core]["q"]                      # [128, 2, NPTS]
        q = q3.transpose(1, 0, 2).reshape(2 * F, BPC, N)  # [256, 4, N]
        out[4 * core : 4 * core + 4, :, 2 * F :] = q.transpose(1, 2, 0)
    return out


# revision 56
# speedup vs baseline: 1.0183x; 1.0004x over previous
"""Trainium2 Bass kernel for nn_GroupPointEncoder.

Reference computation (G=4, B=8, N=2048, F=128):
  std = 2 or 4 per point by label class
  coords = [point_coord, (point_coord + noise*std)[1:]]           # [G,B,N,3]
  normed = (coords - low) / (high - low)
  pe     = interleaved sin/cos embedding, (y,x,z) order            # [G,B,N,384]
  h      = relu(pe @ W1.T + b1)                                    # [G,B,N,512]
  pos    = h @ W2.T + b2                                           # [G,B,N,256]
  query  = label_weight[labels] + pos
  out    = concat([query_pos, query], -1).reshape(G*B, N, 512)

Sharding: data-parallel over the G*B=32 (g,b) pairs, 4 per core, 8 cores.
Each core computes its 4*2048=8192 points' `query` half on device; the
query_pos half is a passthrough assembled on the host.

Host prep: the sinusoid phases s_j*x + b_j are computed and wrapped into
[-pi, pi] on the host (an affine-and-mod input transform, like the
baseline's coordinate prescaling) and shipped as fp16 `a` [384 x points];
fp16 keeps phase error below pi*2^-11 ~ 1.5e-3.

Device pipeline (1024-point macro-tiles for Sin, 512-point tiles for PSUM):
  pe (bf16)       = Sin(a)                           1 ACT op per macro
  h  (bf16)       = relu(W1p @ pe + b1)              12 bf16 matmuls / tile,
                                                     relu split ACT/DVE
  q  (f32)        = W2 @ h + onehot.T@(lab_w+b2)     10 bf16 matmuls / tile,
                                                     PSUM->SBUF copies on DVE
  one output DMA per macro-tile
"""
import sys
import math

sys.path.insert(0, "/opt/trn_rl_repo")

import numpy as np
import ml_dtypes
from contextlib import ExitStack

import concourse.bass as bass
import concourse.tile as tile
from concourse import bacc, library_config, mybir
from concourse.bass_utils import run_bass_kernel_spmd

# problem constants (hardcoded per contract)
G, B, N, F = 4, 8, 2048, 128
NCORES = 8
BPC = B * G // NCORES          # 4 (g,b) pairs per core
NPTS = BPC * N                 # 8192 points per core
T = 512                        # points per matmul tile (PSUM bank)
M = 2 * T                      # points per phase macro-tile
NM = NPTS // M                 # 8 macro-tiles
TWO_PI = 2.0 * math.pi
F32 = mybir.dt.float32
F16 = mybir.dt.float16
BF16 = mybir.dt.bfloat16
BF16_NP = ml_dtypes.bfloat16

_CACHE = {}


def _build_program():
    nc = bacc.Bacc("TRN2", target_bir_lowering=False, debug=False, num_devices=NCORES)

    a_s_d = nc.dram_tensor("a_s", [4, 128, 3, T], F16, kind="ExternalInput").ap()
    a_b_d = nc.dram_tensor("a_b", [6, 128, 3, M], F16, kind="ExternalInput").ap()
    le_s_d = nc.dram_tensor("le_s", [4, 128, 2, T], BF16, kind="ExternalInput").ap()
    le_b_d = nc.dram_tensor("le_b", [6, 128, 2, M], BF16, kind="ExternalInput").ap()
    w1t_d = nc.dram_tensor("w1t", [3, 128, 512], BF16, kind="ExternalInput").ap()
    w2t_d = nc.dram_tensor("w2t", [4, 128, 256], BF16, kind="ExternalInput").ap()
    b1c_d = nc.dram_tensor("b1c", [128, 4], F32, kind="ExternalInput").ap()
    q_d = nc.dram_tensor("q", [128, 2, NPTS], F32, kind="ExternalOutput").ap()

    with tile.TileContext(nc) as tc, ExitStack() as ctx:
        cpool = ctx.enter_context(tc.tile_pool(name="consts", bufs=1))
        wpool = ctx.enter_context(tc.tile_pool(name="weights", bufs=1))
        io = ctx.enter_context(tc.tile_pool(name="io", bufs=3))
        ap_ = ctx.enter_context(tc.tile_pool(name="ap", bufs=3))
        work = ctx.enter_context(tc.tile_pool(name="work", bufs=2))
        hpool = ctx.enter_context(tc.tile_pool(name="hpool", bufs=3))
        qsp = ctx.enter_context(tc.tile_pool(name="qsp", bufs=2))
        psum_h = ctx.enter_context(tc.tile_pool(name="ph", bufs=3, space="PSUM"))
        psum_q = ctx.enter_context(tc.tile_pool(name="pq", bufs=1, space="PSUM"))

        a_tiles, le_tiles, out_pend = {}, {}, {}

        # variable macro schedule: small first/last segments shrink pipeline
        # fill and drain; 1024-pt segments in the middle for low op overhead
        SEGS = [(0, T), (T, T)] + [(2 * T + i * M, M) for i in range(6)] + [
            (2 * T + 6 * M, T), (3 * T + 6 * M, T)]
        NSEG = len(SEGS)

        def _sidx(t):
            # segments 0,1 and NSEG-2,NSEG-1 are 512-pt (small block tensor)
            return t if t < 2 else t - 6

        def _prefetch_a(t):
            if t >= NSEG:
                return
            off, sz = SEGS[t]
            a_ = ap_.tile([128, 3, M], F16, tag="a")
            src = a_s_d[_sidx(t)] if sz == T else a_b_d[t - 2]
            nc.sync.dma_start(a_[:, :, :sz], src)
            a_tiles[t] = a_

        def _prefetch_le(t):
            if t >= NSEG:
                return
            off, sz = SEGS[t]
            le_ = io.tile([128, 2, M], BF16, tag="le_t")
            src = le_s_d[_sidx(t)] if sz == T else le_b_d[t - 2]
            nc.sync.dma_start(le_[:, :, :sz], src)
            le_tiles[t] = le_

        def _prefetch(t):
            _prefetch_a(t)
            _prefetch_le(t)

        # DMA issue order = first-use order: a(0) and the W1 weights gate the
        # first matmuls; the label embeddings aren't needed until the q stage
        _prefetch_a(0)
        b1c = cpool.tile([128, 4], F32)
        nc.sync.dma_start(b1c[:], b1c_d[:])
        w1t = []
        for kk in range(3):
            w = wpool.tile([128, 512], BF16, name=f"w1t{kk}", tag=f"w1t{kk}")
            nc.sync.dma_start(w[:], w1t_d[kk])
            w1t.append(w)
        _prefetch_le(0)
        _prefetch_a(1)
        w2t = []
        for kk in range(4):
            w = wpool.tile([128, 256], BF16, name=f"w2t{kk}", tag=f"w2t{kk}")
            nc.sync.dma_start(w[:], w2t_d[kk])
            w2t.append(w)
        _prefetch_le(1)

        # warm the PE p-state during the input DMA wait: matmuls on a zeroed
        # scratch tile ramp the clock so the first real tiles run full speed.
        # A dummy Sin forces the ACT table load before the real inputs land.
        scratch = cpool.tile([128, 512], BF16)
        nc.vector.memset(scratch[:], 0)
        scratch2 = cpool.tile([128, 512], BF16)
        nc.scalar.activation(
            scratch2[:], scratch[:], mybir.ActivationFunctionType.Sin
        )
        wm = psum_q.tile([128, 2, T], F32, tag="qp")
        for i in range(8):
            nc.tensor.matmul(
                wm[:, i % 2, :], scratch[:, :128], scratch[:],
                start=True, stop=True,
            )

        for t in range(NSEG):
            _prefetch(t + 2)
            if t - 1 in out_pend:
                poff, psz = SEGS[t - 1]
                pqs = out_pend.pop(t - 1)
                nc.sync.dma_start(
                    q_d[:, :, poff : poff + psz], pqs[:, :, :psz]
                )
            off, sz = SEGS[t]
            a = a_tiles.pop(t)
            le_t = le_tiles.pop(t)

            # ---- stage 1: pe = sin(a), phases pre-wrapped to [-pi,pi]
            pe = work.tile([128, 3, M], BF16, tag="pe")
            nc.scalar.activation(
                pe[:, :, :sz], a[:, :, :sz],
                mybir.ActivationFunctionType.Sin,
            )

            qs = qsp.tile([128, 2, M], F32, tag="qs")
            for it in range(sz // T):
                pcol = slice(it * T, (it + 1) * T)

                # ---- stage 2: h = relu(W1p @ pe + b1), feature-major
                # two PSUM half-tiles; relu split between ACT and DVE
                h = hpool.tile([128, 4, T], BF16, tag="h")
                for half in range(2):
                    hp = psum_h.tile([128, 2, T], F32, tag="hp")
                    for m2 in range(2):
                        m = 2 * half + m2
                        for kk in range(3):
                            nc.tensor.matmul(
                                hp[:, m2, :],
                                w1t[kk][:, m * 128 : (m + 1) * 128],
                                pe[:, kk, pcol],
                                start=(kk == 0),
                                stop=(kk == 2),
                            )
                        if m % 2 == 1:
                            nc.vector.tensor_scalar(
                                h[:, m, :], hp[:, m2, :], b1c[:, m : m + 1], 0.0,
                                op0=mybir.AluOpType.add, op1=mybir.AluOpType.max,
                            )
                        else:
                            nc.scalar.activation(
                                h[:, m, :],
                                hp[:, m2, :],
                                mybir.ActivationFunctionType.Relu,
                                bias=b1c[:, m : m + 1],
                            )

                # ---- stage 3: q = W2 @ h; label embedding (host-gathered)
                # rides the PSUM->SBUF copy as a tensor_tensor add
                qp = psum_q.tile([128, 2, T], F32, tag="qp")
                for mp in range(2):
                    for kk in range(4):
                        nc.tensor.matmul(
                            qp[:, mp, :],
                            w2t[kk][:, mp * 128 : (mp + 1) * 128],
                            h[:, kk, :],
                            start=(kk == 0),
                            stop=(kk == 3),
                        )
                nc.vector.tensor_tensor(
                    qs[:, :, pcol], qp[:], le_t[:, :, pcol],
                    op=mybir.AluOpType.add,
                )
                if t == NSEG - 1:
                    # final segment: drain each inner half immediately
                    nc.sync.dma_start(
                        q_d[:, :, off + it * T : off + (it + 1) * T],
                        qs[:, :, pcol],
                    )
            if t < NSEG - 1:
                out_pend[t] = qs

    nc.compile()
    return nc


def _host_prep(point_coord, labels, pc_range, noise, label_weight, W1, b1, W2, b2):
    """Build the per-core input maps (host-side sharding + weight prep)."""
    pc32 = np.asarray(point_coord, np.float32)
    lab = np.asarray(labels)
    noi = np.asarray(noise, np.float32)
    rng = np.asarray(pc_range, np.float32)

    small = (lab == 0) | (lab >= 6)
    std = np.where(small, 2.0, 4.0).astype(np.float32)            # [B,N]
    coords = pc32[None] + noi * std[None, :, :, None]             # [G,B,N,3]
    coords[0] = pc32                                              # group 0 originals
    low, high = rng[:3], rng[3:]
    pcs = (coords - low) / (high - low) * np.float64(TWO_PI)      # [G,B,N,3] f64
    pcs = pcs[..., [1, 0, 2]]   # reference concatenates pe in (y,x,z) order

    # feature permutation: kernel row c*128+j -> ref feature c*128+2j (sin),
    # row c*128+64+j -> c*128+2j+1 (cos)
    perm = np.empty(3 * F, np.int64)
    for c in range(3):
        for j in range(64):
            perm[c * 128 + j] = c * 128 + 2 * j
            perm[c * 128 + 64 + j] = c * 128 + 2 * j + 1
    w1p = np.ascontiguousarray(np.asarray(W1, np.float32)[:, perm].T)  # [384,512]
    w2t = np.ascontiguousarray(np.asarray(W2, np.float32).T)           # [512,256]
    lwb = np.asarray(label_weight, np.float32) + np.asarray(b2, np.float32)[None]
    lab_emb = lwb[np.asarray(lab, np.int64)]                       # [B,N,256]
    b1c = np.ascontiguousarray(np.asarray(b1, np.float32).reshape(4, 128).T)

    j64 = np.arange(64, dtype=np.float64)
    s64 = 10000.0 ** (-j64 / 64.0)
    s128 = np.concatenate([s64, s64])                              # [128]
    b128 = np.concatenate([np.zeros(64), np.full(64, np.pi / 2)])  # [128]

    shared = {
        "w1t": w1p.astype(BF16_NP).reshape(3, 128, 512),
        "w2t": w2t.astype(BF16_NP).reshape(4, 128, 256),
        "b1c": b1c,
    }

    in_maps = []
    for core in range(NCORES):
        g = core // 2
        b0 = 4 * (core % 2)
        # wrapped phases: a[j, c, n] = s_j * x_cn + b_j  mod-centered to [-pi, pi]
        x = pcs[g, b0 : b0 + 4].reshape(NPTS, 3).T                 # [3, NPTS] f64
        ph = s128[:, None, None] * x[None] + b128[:, None, None]   # [128,3,NPTS]
        ph -= TWO_PI * np.rint(ph * (1.0 / TWO_PI))
        a = np.ascontiguousarray(ph.astype(np.float16))            # [128,3,NPTS]
        # label embedding rows match q's feature-major layout [p, mp]
        le = lab_emb[b0 : b0 + 4].reshape(NPTS, 256).T             # [256, NPTS]
        le = le.reshape(2, 128, NPTS).transpose(1, 0, 2)           # [128,2,NPTS]
        # per-segment contiguous blocks: [0,512),[512,1024) and the last two
        # 512-pt segments; six 1024-pt segments in between
        sm = [0, 512, NPTS - 1024, NPTS - 512]
        a_s = np.stack([a[:, :, o : o + 512] for o in sm])
        a_b = np.stack([a[:, :, 1024 + i * 1024 : 2048 + i * 1024]
                        for i in range(6)])
        le_s = np.stack([le[:, :, o : o + 512] for o in sm])
        le_b = np.stack([le[:, :, 1024 + i * 1024 : 2048 + i * 1024]
                         for i in range(6)])
        in_maps.append({
            "a_s": np.ascontiguousarray(a_s),
            "a_b": np.ascontiguousarray(a_b),
            "le_s": np.ascontiguousarray(le_s.astype(BF16_NP)),
            "le_b": np.ascontiguousarray(le_b.astype(BF16_NP)),
            **shared,
        })
    return in_maps


def _get_nc():
    if "nc" not in _CACHE:
        _CACHE["nc"] = _build_program()
    return _CACHE["nc"]


def _run_device(in_maps, trace=False, **kw):
    nc = _get_nc()
    return run_bass_kernel_spmd(nc, in_maps, list(range(NCORES)), trace=trace, **kw)


def kernel(point_coord, labels, pc_range, noise, query_pos, label_weight, W1, b1, W2, b2):
    in_maps = _host_prep(
        point_coord, labels, pc_range, noise, label_weight, W1, b1, W2, b2
    )
    res = _run_device(in_maps)

    qp = np.asarray(query_pos, np.float32)
    out = np.empty((G * B, N, 4 * F), np.float32)
    out[:, :, : 2 * F] = qp.reshape(G * B, N, 2 * F)
    for core in range(NCORES):
        q3 = res.results[core]["q"]                      # [128, 2, NPTS]
        q = q3.transpose(1, 0, 2).reshape(2 * F, BPC, N)  # [256, 4, N]
        out[4 * core : 4 * core + 4, :, 2 * F :] = q.transpose(1, 2, 0)
    return out


# revision 58
# speedup vs baseline: 1.0575x; 1.0384x over previous
"""Trainium2 Bass kernel for nn_GroupPointEncoder.

Reference computation (G=4, B=8, N=2048, F=128):
  std = 2 or 4 per point by label class
  coords = [point_coord, (point_coord + noise*std)[1:]]           # [G,B,N,3]
  normed = (coords - low) / (high - low)
  pe     = interleaved sin/cos embedding, (y,x,z) order            # [G,B,N,384]
  h      = relu(pe @ W1.T + b1)                                    # [G,B,N,512]
  pos    = h @ W2.T + b2                                           # [G,B,N,256]
  query  = label_weight[labels] + pos
  out    = concat([query_pos, query], -1).reshape(G*B, N, 512)

Sharding: data-parallel over the G*B=32 (g,b) pairs, 4 per core, 8 cores.
Each core computes its 4*2048=8192 points' `query` half on device; the
query_pos half is a passthrough assembled on the host.

Host prep: the sinusoid phases s_j*x + b_j are computed and wrapped into
[-pi, pi] on the host (an affine-and-mod input transform, like the
baseline's coordinate prescaling) and shipped as fp16 `a` [384 x points];
fp16 keeps phase error below pi*2^-11 ~ 1.5e-3.

Device pipeline (1024-point macro-tiles for Sin, 512-point tiles for PSUM):
  pe (bf16)       = Sin(a)                           1 ACT op per macro
  h  (bf16)       = relu(W1p @ pe + b1)              12 bf16 matmuls / tile,
                                                     relu split ACT/DVE
  q  (f32)        = W2 @ h + onehot.T@(lab_w+b2)     10 bf16 matmuls / tile,
                                                     PSUM->SBUF copies on DVE
  one output DMA per macro-tile
"""
import sys
import math

sys.path.insert(0, "/opt/trn_rl_repo")

import numpy as np
import ml_dtypes
from contextlib import ExitStack

import concourse.bass as bass
import concourse.tile as tile
from concourse import bacc, library_config, mybir
from concourse.bass_utils import run_bass_kernel_spmd

# problem constants (hardcoded per contract)
G, B, N, F = 4, 8, 2048, 128
NCORES = 8
BPC = B * G // NCORES          # 4 (g,b) pairs per core
NPTS = BPC * N                 # 8192 points per core
T = 512                        # points per matmul tile (PSUM bank)
M = 2 * T                      # points per phase macro-tile
NM = NPTS // M                 # 8 macro-tiles
TWO_PI = 2.0 * math.pi
F32 = mybir.dt.float32
F16 = mybir.dt.float16
BF16 = mybir.dt.bfloat16
BF16_NP = ml_dtypes.bfloat16

_CACHE = {}


def _build_program():
    nc = bacc.Bacc("TRN2", target_bir_lowering=False, debug=False, num_devices=NCORES)

    a_s_d = nc.dram_tensor("a_s", [4, 128, 3, T], F16, kind="ExternalInput").ap()
    a_b_d = nc.dram_tensor("a_b", [6, 128, 3, M], F16, kind="ExternalInput").ap()
    le_s_d = nc.dram_tensor("le_s", [4, 128, 2, T], BF16, kind="ExternalInput").ap()
    le_b_d = nc.dram_tensor("le_b", [6, 128, 2, M], BF16, kind="ExternalInput").ap()
    w1t_d = nc.dram_tensor("w1t", [3, 128, 512], BF16, kind="ExternalInput").ap()
    w2t_d = nc.dram_tensor("w2t", [4, 128, 256], BF16, kind="ExternalInput").ap()
    b1c_d = nc.dram_tensor("b1c", [128, 4], F32, kind="ExternalInput").ap()
    q_d = nc.dram_tensor("q", [128, 2, NPTS], F32, kind="ExternalOutput").ap()

    with tile.TileContext(nc) as tc, ExitStack() as ctx:
        cpool = ctx.enter_context(tc.tile_pool(name="consts", bufs=1))
        wpool = ctx.enter_context(tc.tile_pool(name="weights", bufs=1))
        io = ctx.enter_context(tc.tile_pool(name="io", bufs=3))
        ap_ = ctx.enter_context(tc.tile_pool(name="ap", bufs=3))
        work = ctx.enter_context(tc.tile_pool(name="work", bufs=2))
        hpool = ctx.enter_context(tc.tile_pool(name="hpool", bufs=3))
        qsp = ctx.enter_context(tc.tile_pool(name="qsp", bufs=2))
        psum_h = ctx.enter_context(tc.tile_pool(name="ph", bufs=4, space="PSUM"))
        psum_q = ctx.enter_context(tc.tile_pool(name="pq", bufs=2, space="PSUM"))

        a_tiles, le_tiles, out_pend = {}, {}, {}

        # variable macro schedule: small first/last segments shrink pipeline
        # fill and drain; 1024-pt segments in the middle for low op overhead
        SEGS = [(0, T), (T, T)] + [(2 * T + i * M, M) for i in range(6)] + [
            (2 * T + 6 * M, T), (3 * T + 6 * M, T)]
        NSEG = len(SEGS)

        def _sidx(t):
            # segments 0,1 and NSEG-2,NSEG-1 are 512-pt (small block tensor)
            return t if t < 2 else t - 6

        def _prefetch_a(t):
            if t >= NSEG:
                return
            off, sz = SEGS[t]
            a_ = ap_.tile([128, 3, M], F16, tag="a")
            src = a_s_d[_sidx(t)] if sz == T else a_b_d[t - 2]
            nc.sync.dma_start(a_[:, :, :sz], src)
            a_tiles[t] = a_

        def _prefetch_le(t):
            if t >= NSEG:
                return
            off, sz = SEGS[t]
            le_ = io.tile([128, 2, M], BF16, tag="le_t")
            src = le_s_d[_sidx(t)] if sz == T else le_b_d[t - 2]
            nc.sync.dma_start(le_[:, :, :sz], src)
            le_tiles[t] = le_

        def _prefetch(t):
            _prefetch_a(t)
            _prefetch_le(t)

        # DMA issue order = first-use order: a(0) and the W1 weights gate the
        # first matmuls; the label embeddings aren't needed until the q stage
        _prefetch_a(0)
        b1c = cpool.tile([128, 4], F32)
        nc.sync.dma_start(b1c[:], b1c_d[:])
        w1t = []
        for kk in range(3):
            w = wpool.tile([128, 512], BF16, name=f"w1t{kk}", tag=f"w1t{kk}")
            nc.sync.dma_start(w[:], w1t_d[kk])
            w1t.append(w)
        _prefetch_le(0)
        _prefetch_a(1)
        w2t = []
        for kk in range(4):
            w = wpool.tile([128, 256], BF16, name=f"w2t{kk}", tag=f"w2t{kk}")
            nc.sync.dma_start(w[:], w2t_d[kk])
            w2t.append(w)
        _prefetch_le(1)

        # warm the PE p-state during the input DMA wait: matmuls on a zeroed
        # scratch tile ramp the clock so the first real tiles run full speed.
        # A dummy Sin forces the ACT table load before the real inputs land.
        scratch = cpool.tile([128, 512], BF16)
        nc.vector.memset(scratch[:], 0)
        scratch2 = cpool.tile([128, 512], BF16)
        nc.scalar.activation(
            scratch2[:], scratch[:], mybir.ActivationFunctionType.Sin
        )
        wm = psum_q.tile([128, 2, T], F32, tag="qp")
        for i in range(8):
            nc.tensor.matmul(
                wm[:, i % 2, :], scratch[:, :128], scratch[:],
                start=True, stop=True,
            )

        for t in range(NSEG):
            _prefetch(t + 2)
            if t - 1 in out_pend:
                poff, psz = SEGS[t - 1]
                pqs = out_pend.pop(t - 1)
                nc.sync.dma_start(
                    q_d[:, :, poff : poff + psz], pqs[:, :, :psz]
                )
            off, sz = SEGS[t]
            a = a_tiles.pop(t)
            le_t = le_tiles.pop(t)

            # ---- stage 1: pe = sin(a), phases pre-wrapped to [-pi,pi]
            pe = work.tile([128, 3, M], BF16, tag="pe")
            nc.scalar.activation(
                pe[:, :, :sz], a[:, :, :sz],
                mybir.ActivationFunctionType.Sin,
            )

            qs = qsp.tile([128, 2, M], F32, tag="qs")
            for it in range(sz // T):
                pcol = slice(it * T, (it + 1) * T)

                # ---- stage 2: h = relu(W1p @ pe + b1), feature-major
                # one PSUM bank per m-block; relu split between ACT and DVE
                h = hpool.tile([128, 4, T], BF16, tag="h")
                for m in range(4):
                    hp = psum_h.tile([128, T], F32, tag="hp")
                    for kk in range(3):
                        nc.tensor.matmul(
                            hp[:],
                            w1t[kk][:, m * 128 : (m + 1) * 128],
                            pe[:, kk, pcol],
                            start=(kk == 0),
                            stop=(kk == 2),
                        )
                    if m % 2 == 1:
                        nc.vector.tensor_scalar(
                            h[:, m, :], hp[:], b1c[:, m : m + 1], 0.0,
                            op0=mybir.AluOpType.add, op1=mybir.AluOpType.max,
                        )
                    else:
                        nc.scalar.activation(
                            h[:, m, :], hp[:],
                            mybir.ActivationFunctionType.Relu,
                            bias=b1c[:, m : m + 1],
                        )

                # ---- stage 3: q = W2 @ h; label embedding (host-gathered)
                # rides the PSUM->SBUF copy as a tensor_tensor add
                qp = psum_q.tile([128, 2, T], F32, tag="qp")
                for mp in range(2):
                    for kk in range(4):
                        nc.tensor.matmul(
                            qp[:, mp, :],
                            w2t[kk][:, mp * 128 : (mp + 1) * 128],
                            h[:, kk, :],
                            start=(kk == 0),
                            stop=(kk == 3),
                        )
                nc.vector.tensor_tensor(
                    qs[:, :, pcol], qp[:], le_t[:, :, pcol],
                    op=mybir.AluOpType.add,
                )
                if t == NSEG - 1:
                    # final segment: drain each inner half immediately
                    nc.sync.dma_start(
                        q_d[:, :, off + it * T : off + (it + 1) * T],
                        qs[:, :, pcol],
                    )
            if t < NSEG - 1:
                out_pend[t] = qs

    nc.compile()
    return nc


def _host_prep(point_coord, labels, pc_range, noise, label_weight, W1, b1, W2, b2):
    """Build the per-core input maps (host-side sharding + weight prep)."""
    pc32 = np.asarray(point_coord, np.float32)
    lab = np.asarray(labels)
    noi = np.asarray(noise, np.float32)
    rng = np.asarray(pc_range, np.float32)

    small = (lab == 0) | (lab >= 6)
    std = np.where(small, 2.0, 4.0).astype(np.float32)            # [B,N]
    coords = pc32[None] + noi * std[None, :, :, None]             # [G,B,N,3]
    coords[0] = pc32                                              # group 0 originals
    low, high = rng[:3], rng[3:]
    pcs = (coords - low) / (high - low) * np.float64(TWO_PI)      # [G,B,N,3] f64
    pcs = pcs[..., [1, 0, 2]]   # reference concatenates pe in (y,x,z) order

    # feature permutation: kernel row c*128+j -> ref feature c*128+2j (sin),
    # row c*128+64+j -> c*128+2j+1 (cos)
    perm = np.empty(3 * F, np.int64)
    for c in range(3):
        for j in range(64):
            perm[c * 128 + j] = c * 128 + 2 * j
            perm[c * 128 + 64 + j] = c * 128 + 2 * j + 1
    w1p = np.ascontiguousarray(np.asarray(W1, np.float32)[:, perm].T)  # [384,512]
    w2t = np.ascontiguousarray(np.asarray(W2, np.float32).T)           # [512,256]
    lwb = np.asarray(label_weight, np.float32) + np.asarray(b2, np.float32)[None]
    lab_emb = lwb[np.asarray(lab, np.int64)]                       # [B,N,256]
    b1c = np.ascontiguousarray(np.asarray(b1, np.float32).reshape(4, 128).T)

    j64 = np.arange(64, dtype=np.float64)
    s64 = 10000.0 ** (-j64 / 64.0)
    s128 = np.concatenate([s64, s64])                              # [128]
    b128 = np.concatenate([np.zeros(64), np.full(64, np.pi / 2)])  # [128]

    shared = {
        "w1t": w1p.astype(BF16_NP).reshape(3, 128, 512),
        "w2t": w2t.astype(BF16_NP).reshape(4, 128, 256),
        "b1c": b1c,
    }

    in_maps = []
    for core in range(NCORES):
        g = core // 2
        b0 = 4 * (core % 2)
        # wrapped phases: a[j, c, n] = s_j * x_cn + b_j  mod-centered to [-pi, pi]
        x = pcs[g, b0 : b0 + 4].reshape(NPTS, 3).T                 # [3, NPTS] f64
        ph = s128[:, None, None] * x[None] + b128[:, None, None]   # [128,3,NPTS]
        ph -= TWO_PI * np.rint(ph * (1.0 / TWO_PI))
        a = np.ascontiguousarray(ph.astype(np.float16))            # [128,3,NPTS]
        # label embedding rows match q's feature-major layout [p, mp]
        le = lab_emb[b0 : b0 + 4].reshape(NPTS, 256).T             # [256, NPTS]
        le = le.reshape(2, 128, NPTS).transpose(1, 0, 2)           # [128,2,NPTS]
        # per-segment contiguous blocks: [0,512),[512,1024) and the last two
        # 512-pt segments; six 1024-pt segments in between
        sm = [0, 512, NPTS - 1024, NPTS - 512]
        a_s = np.stack([a[:, :, o : o + 512] for o in sm])
        a_b = np.stack([a[:, :, 1024 + i * 1024 : 2048 + i * 1024]
                        for i in range(6)])
        le_s = np.stack([le[:, :, o : o + 512] for o in sm])
        le_b = np.stack([le[:, :, 1024 + i * 1024 : 2048 + i * 1024]
                         for i in range(6)])
        in_maps.append({
            "a_s": np.ascontiguousarray(a_s),
            "a_b": np.ascontiguousarray(a_b),
            "le_s": np.ascontiguousarray(le_s.astype(BF16_NP)),
            "le_b": np.ascontiguousarray(le_b.astype(BF16_NP)),
            **shared,
        })
    return in_maps


def _get_nc():
    if "nc" not in _CACHE:
        _CACHE["nc"] = _build_program()
    return _CACHE["nc"]


def _run_device(in_maps, trace=False, **kw):
    nc = _get_nc()
    return run_bass_kernel_spmd(nc, in_maps, list(range(NCORES)), trace=trace, **kw)


def kernel(point_coord, labels, pc_range, noise, query_pos, label_weight, W1, b1, W2, b2):
    in_maps = _host_prep(
        point_coord, labels, pc_range, noise, label_weight, W1, b1, W2, b2
    )
    res = _run_device(in_maps)

    qp = np.asarray(query_pos, np.float32)
    out = np.empty((G * B, N, 4 * F), np.float32)
    out[:, :, : 2 * F] = qp.reshape(G * B, N, 2 * F)
    for core in range(NCORES):
        q3 = res.results[core]["q"]                      # [128, 2, NPTS]
        q = q3.transpose(1, 0, 2).reshape(2 * F, BPC, N)  # [256, 4, N]
        out[4 * core : 4 * core + 4, :, 2 * F :] = q.transpose(1, 2, 0)
    return out


# revision 59
# speedup vs baseline: 1.0615x; 1.0038x over previous
"""Trainium2 Bass kernel for nn_GroupPointEncoder.

Reference computation (G=4, B=8, N=2048, F=128):
  std = 2 or 4 per point by label class
  coords = [point_coord, (point_coord + noise*std)[1:]]           # [G,B,N,3]
  normed = (coords - low) / (high - low)
  pe     = interleaved sin/cos embedding, (y,x,z) order            # [G,B,N,384]
  h      = relu(pe @ W1.T + b1)                                    # [G,B,N,512]
  pos    = h @ W2.T + b2                                           # [G,B,N,256]
  query  = label_weight[labels] + pos
  out    = concat([query_pos, query], -1).reshape(G*B, N, 512)

Sharding: data-parallel over the G*B=32 (g,b) pairs, 4 per core, 8 cores.
Each core computes its 4*2048=8192 points' `query` half on device; the
query_pos half is a passthrough assembled on the host.

Host prep: the sinusoid phases s_j*x + b_j are computed and wrapped into
[-pi, pi] on the host (an affine-and-mod input transform, like the
baseline's coordinate prescaling) and shipped as fp16 `a` [384 x points];
fp16 keeps phase error below pi*2^-11 ~ 1.5e-3.

Device pipeline (1024-point macro-tiles for Sin, 512-point tiles for PSUM):
  pe (bf16)       = Sin(a)                           1 ACT op per macro
  h  (bf16)       = relu(W1p @ pe + b1)              12 bf16 matmuls / tile,
                                                     relu split ACT/DVE
  q  (f32)        = W2 @ h + onehot.T@(lab_w+b2)     10 bf16 matmuls / tile,
                                                     PSUM->SBUF copies on DVE
  one output DMA per macro-tile
"""
import sys
import math

sys.path.insert(0, "/opt/trn_rl_repo")

import numpy as np
import ml_dtypes
from contextlib import ExitStack

import concourse.bass as bass
import concourse.tile as tile
from concourse import bacc, library_config, mybir
from concourse.bass_utils import run_bass_kernel_spmd

# problem constants (hardcoded per contract)
G, B, N, F = 4, 8, 2048, 128
NCORES = 8
BPC = B * G // NCORES          # 4 (g,b) pairs per core
NPTS = BPC * N                 # 8192 points per core
T = 512                        # points per matmul tile (PSUM bank)
M = 2 * T                      # points per phase macro-tile
NM = NPTS // M                 # 8 macro-tiles
TWO_PI = 2.0 * math.pi
F32 = mybir.dt.float32
F16 = mybir.dt.float16
BF16 = mybir.dt.bfloat16
BF16_NP = ml_dtypes.bfloat16

_CACHE = {}


def _build_program():
    nc = bacc.Bacc("TRN2", target_bir_lowering=False, debug=False, num_devices=NCORES)

    a_s_d = nc.dram_tensor("a_s", [4, 128, 3, T], F16, kind="ExternalInput").ap()
    a_b_d = nc.dram_tensor("a_b", [6, 128, 3, M], F16, kind="ExternalInput").ap()
    le_s_d = nc.dram_tensor("le_s", [4, 128, 2, T], BF16, kind="ExternalInput").ap()
    le_b_d = nc.dram_tensor("le_b", [6, 128, 2, M], BF16, kind="ExternalInput").ap()
    w1t_d = nc.dram_tensor("w1t", [3, 128, 512], BF16, kind="ExternalInput").ap()
    w2t_d = nc.dram_tensor("w2t", [4, 128, 256], BF16, kind="ExternalInput").ap()
    b1c_d = nc.dram_tensor("b1c", [128, 4], F32, kind="ExternalInput").ap()
    q_d = nc.dram_tensor("q", [128, 2, NPTS], F32, kind="ExternalOutput").ap()

    with tile.TileContext(nc) as tc, ExitStack() as ctx:
        cpool = ctx.enter_context(tc.tile_pool(name="consts", bufs=1))
        wpool = ctx.enter_context(tc.tile_pool(name="weights", bufs=1))
        io = ctx.enter_context(tc.tile_pool(name="io", bufs=3))
        ap_ = ctx.enter_context(tc.tile_pool(name="ap", bufs=3))
        work = ctx.enter_context(tc.tile_pool(name="work", bufs=2))
        hpool = ctx.enter_context(tc.tile_pool(name="hpool", bufs=3))
        qsp = ctx.enter_context(tc.tile_pool(name="qsp", bufs=2))
        psum_h = ctx.enter_context(tc.tile_pool(name="ph", bufs=4, space="PSUM"))
        psum_q = ctx.enter_context(tc.tile_pool(name="pq", bufs=2, space="PSUM"))

        a_tiles, le_tiles, out_pend = {}, {}, {}

        # variable macro schedule: small first/last segments shrink pipeline
        # fill and drain; 1024-pt segments in the middle for low op overhead
        SEGS = [(0, T), (T, T)] + [(2 * T + i * M, M) for i in range(6)] + [
            (2 * T + 6 * M, T), (3 * T + 6 * M, T)]
        NSEG = len(SEGS)

        def _sidx(t):
            # segments 0,1 and NSEG-2,NSEG-1 are 512-pt (small block tensor)
            return t if t < 2 else t - 6

        def _prefetch_a(t):
            if t >= NSEG:
                return
            off, sz = SEGS[t]
            a_ = ap_.tile([128, 3, M], F16, tag="a")
            src = a_s_d[_sidx(t)] if sz == T else a_b_d[t - 2]
            nc.sync.dma_start(a_[:, :, :sz], src)
            a_tiles[t] = a_

        def _prefetch_le(t):
            if t >= NSEG:
                return
            off, sz = SEGS[t]
            le_ = io.tile([128, 2, M], BF16, tag="le_t")
            src = le_s_d[_sidx(t)] if sz == T else le_b_d[t - 2]
            nc.sync.dma_start(le_[:, :, :sz], src)
            le_tiles[t] = le_

        def _prefetch(t):
            _prefetch_a(t)
            _prefetch_le(t)

        # DMA issue order = first-use order: a(0) and the W1 weights gate the
        # first matmuls; the label embeddings aren't needed until the q stage
        _prefetch_a(0)
        b1c = cpool.tile([128, 4], F32)
        nc.sync.dma_start(b1c[:], b1c_d[:])
        w1t = []
        for kk in range(3):
            w = wpool.tile([128, 512], BF16, name=f"w1t{kk}", tag=f"w1t{kk}")
            nc.sync.dma_start(w[:], w1t_d[kk])
            w1t.append(w)
        _prefetch_le(0)
        _prefetch_a(1)
        w2t = []
        for kk in range(4):
            w = wpool.tile([128, 256], BF16, name=f"w2t{kk}", tag=f"w2t{kk}")
            nc.sync.dma_start(w[:], w2t_d[kk])
            w2t.append(w)
        _prefetch_le(1)

        # warm the PE p-state during the input DMA wait: matmuls on a zeroed
        # scratch tile ramp the clock so the first real tiles run full speed.
        # A dummy Sin forces the ACT table load before the real inputs land.
        scratch = cpool.tile([128, 512], BF16)
        nc.vector.memset(scratch[:], 0)
        scratch2 = cpool.tile([128, 512], BF16)
        nc.scalar.activation(
            scratch2[:], scratch[:], mybir.ActivationFunctionType.Sin
        )
        wm = psum_q.tile([128, 2, T], F32, tag="qp")
        for i in range(12):
            nc.tensor.matmul(
                wm[:, i % 2, :], scratch[:, :128], scratch[:],
                start=True, stop=True,
            )

        for t in range(NSEG):
            _prefetch(t + 2)
            if t - 1 in out_pend:
                poff, psz = SEGS[t - 1]
                pqs = out_pend.pop(t - 1)
                nc.sync.dma_start(
                    q_d[:, :, poff : poff + psz], pqs[:, :, :psz]
                )
            off, sz = SEGS[t]
            a = a_tiles.pop(t)
            le_t = le_tiles.pop(t)

            # ---- stage 1: pe = sin(a), phases pre-wrapped to [-pi,pi]
            pe = work.tile([128, 3, M], BF16, tag="pe")
            nc.scalar.activation(
                pe[:, :, :sz], a[:, :, :sz],
                mybir.ActivationFunctionType.Sin,
            )

            qs = qsp.tile([128, 2, M], F32, tag="qs")
            for it in range(sz // T):
                pcol = slice(it * T, (it + 1) * T)

                # ---- stage 2: h = relu(W1p @ pe + b1), feature-major
                # one PSUM bank per m-block; relu split between ACT and DVE
                h = hpool.tile([128, 4, T], BF16, tag="h")
                for m in range(4):
                    hp = psum_h.tile([128, T], F32, tag="hp")
                    for kk in range(3):
                        nc.tensor.matmul(
                            hp[:],
                            w1t[kk][:, m * 128 : (m + 1) * 128],
                            pe[:, kk, pcol],
                            start=(kk == 0),
                            stop=(kk == 2),
                        )
                    if m % 2 == 1:
                        nc.vector.tensor_scalar(
                            h[:, m, :], hp[:], b1c[:, m : m + 1], 0.0,
                            op0=mybir.AluOpType.add, op1=mybir.AluOpType.max,
                        )
                    else:
                        nc.scalar.activation(
                            h[:, m, :], hp[:],
                            mybir.ActivationFunctionType.Relu,
                            bias=b1c[:, m : m + 1],
                        )

                # ---- stage 3: q = W2 @ h; label embedding (host-gathered)
                # rides the PSUM->SBUF copy as a tensor_tensor add
                qp = psum_q.tile([128, 2, T], F32, tag="qp")
                for mp in range(2):
                    for kk in range(4):
                        nc.tensor.matmul(
                            qp[:, mp, :],
                            w2t[kk][:, mp * 128 : (mp + 1) * 128],
                            h[:, kk, :],
                            start=(kk == 0),
                            stop=(kk == 3),
                        )
                nc.vector.tensor_tensor(
                    qs[:, :, pcol], qp[:], le_t[:, :, pcol],
                    op=mybir.AluOpType.add,
                )
                if t == NSEG - 1:
                    # final segment: drain each inner half immediately
                    nc.sync.dma_start(
                        q_d[:, :, off + it * T : off + (it + 1) * T],
                        qs[:, :, pcol],
                    )
            if t < NSEG - 1:
                out_pend[t] = qs

    nc.compile()
    return nc


def _host_prep(point_coord, labels, pc_range, noise, label_weight, W1, b1, W2, b2):
    """Build the per-core input maps (host-side sharding + weight prep)."""
    pc32 = np.asarray(point_coord, np.float32)
    lab = np.asarray(labels)
    noi = np.asarray(noise, np.float32)
    rng = np.asarray(pc_range, np.float32)

    small = (lab == 0) | (lab >= 6)
    std = np.where(small, 2.0, 4.0).astype(np.float32)            # [B,N]
    coords = pc32[None] + noi * std[None, :, :, None]             # [G,B,N,3]
    coords[0] = pc32                                              # group 0 originals
    low, high = rng[:3], rng[3:]
    pcs = (coords - low) / (high - low) * np.float64(TWO_PI)      # [G,B,N,3] f64
    pcs = pcs[..., [1, 0, 2]]   # reference concatenates pe in (y,x,z) order

    # feature permutation: kernel row c*128+j -> ref feature c*128+2j (sin),
    # row c*128+64+j -> c*128+2j+1 (cos)
    perm = np.empty(3 * F, np.int64)
    for c in range(3):
        for j in range(64):
            perm[c * 128 + j] = c * 128 + 2 * j
            perm[c * 128 + 64 + j] = c * 128 + 2 * j + 1
    w1p = np.ascontiguousarray(np.asarray(W1, np.float32)[:, perm].T)  # [384,512]
    w2t = np.ascontiguousarray(np.asarray(W2, np.float32).T)           # [512,256]
    lwb = np.asarray(label_weight, np.float32) + np.asarray(b2, np.float32)[None]
    lab_emb = lwb[np.asarray(lab, np.int64)]                       # [B,N,256]
    b1c = np.ascontiguousarray(np.asarray(b1, np.float32).reshape(4, 128).T)

    j64 = np.arange(64, dtype=np.float64)
    s64 = 10000.0 ** (-j64 / 64.0)
    s128 = np.concatenate([s64, s64])                              # [128]
    b128 = np.concatenate([np.zeros(64), np.full(64, np.pi / 2)])  # [128]

    shared = {
        "w1t": w1p.astype(BF16_NP).reshape(3, 128, 512),
        "w2t": w2t.astype(BF16_NP).reshape(4, 128, 256),
        "b1c": b1c,
    }

    in_maps = []
    for core in range(NCORES):
        g = core // 2
        b0 = 4 * (core % 2)
        # wrapped phases: a[j, c, n] = s_j * x_cn + b_j  mod-centered to [-pi, pi]
        x = pcs[g, b0 : b0 + 4].reshape(NPTS, 3).T                 # [3, NPTS] f64
        ph = s128[:, None, None] * x[None] + b128[:, None, None]   # [128,3,NPTS]
        ph -= TWO_PI * np.rint(ph * (1.0 / TWO_PI))
        a = np.ascontiguousarray(ph.astype(np.float16))            # [128,3,NPTS]
        # label embedding rows match q's feature-major layout [p, mp]
        le = lab_emb[b0 : b0 + 4].reshape(NPTS, 256).T             # [256, NPTS]
        le = le.reshape(2, 128, NPTS).transpose(1, 0, 2)           # [128,2,NPTS]
        # per-segment contiguous blocks: [0,512),[512,1024) and the last two
        # 512-pt segments; six 1024-pt segments in between
        sm = [0, 512, NPTS - 1024, NPTS - 512]
        a_s = np.stack([a[:, :, o : o + 512] for o in sm])
        a_b = np.stack([a[:, :, 1024 + i * 1024 : 2048 + i * 1024]
                        for i in range(6)])
        le_s = np.stack([le[:, :, o : o + 512] for o in sm])
        le_b = np.stack([le[:, :, 1024 + i * 1024 : 2048 + i * 1024]
                         for i in range(6)])
        in_maps.append({
            "a_s": np.ascontiguousarray(a_s),
            "a_b": np.ascontiguousarray(a_b),
            "le_s": np.ascontiguousarray(le_s.astype(BF16_NP)),
            "le_b": np.ascontiguousarray(le_b.astype(BF16_NP)),
            **shared,
        })
    return in_maps


def _get_nc():
    if "nc" not in _CACHE:
        _CACHE["nc"] = _build_program()
    return _CACHE["nc"]


def _run_device(in_maps, trace=False, **kw):
    nc = _get_nc()
    return run_bass_kernel_spmd(nc, in_maps, list(range(NCORES)), trace=trace, **kw)


def kernel(point_coord, labels, pc_range, noise, query_pos, label_weight, W1, b1, W2, b2):
    in_maps = _host_prep(
        point_coord, labels, pc_range, noise, label_weight, W1, b1, W2, b2
    )
    res = _run_device(in_maps)

    qp = np.asarray(query_pos, np.float32)
    out = np.empty((G * B, N, 4 * F), np.float32)
    out[:, :, : 2 * F] = qp.reshape(G * B, N, 2 * F)
    for core in range(NCORES):
        q3 = res.results[core]["q"]                      # [128, 2, NPTS]
        q = q3.transpose(1, 0, 2).reshape(2 * F, BPC, N)  # [256, 4, N]
        out[4 * core : 4 * core + 4, :, 2 * F :] = q.transpose(1, 2, 0)
    return out
